# revision 2
# baseline (speedup 1.0000x reference)
# CrossEntropyLoss (ignore_index=0, ragged lengths) for logits [16, 513, 32000] f32.
#
# loss = sum_{valid} (log(sum_v exp(x[r, v])) - x[r, tgt_r]) / n_valid
#   valid = (s < lengths[b]) & (tgt != 0), over rows r = (b, s) with s in [0, 512)
#   (positions are output[:, 1:] / trg[:, 1:])
#
# Strategy: the only heavy work is sum_v exp(x) over the valid rows (~0.5-1 GB
# streamed from HBM).  Host packs just the valid rows, shards them across the
# 8 NeuronCores, and the device kernel computes per-row sum(exp(x)) with the
# ScalarEngine's fused exp+accumulate.  Everything else (target gather, mask,
# log, final divide) is O(B*S) and done on host in float64.

import math

import numpy as np

B, SP1, V = 16, 513, 32000
S = SP1 - 1
N_CORES = 8
P = 128           # SBUF partitions
CHUNK = 4000      # V-chunk width per DMA/ACT op
N_CHUNKS = V // CHUNK

_NC_CACHE: dict = {}


def _build_nc(n_blocks: int, chunk: int = CHUNK, bufs_in: int = 6, bufs_out: int = 3):
    """Bass/Tile kernel: rows = n_blocks*128 rows of V=32000 f32 in DRAM;
    out[b, p, c] = sum over chunk c of exp(x[row, :]) for row = b*128+p."""
    import concourse.bacc as bacc
    import concourse.mybir as mybir
    import concourse.tile as tile

    n_chunks = V // chunk
    key = (n_blocks, chunk, bufs_in, bufs_out)
    if key in _NC_CACHE:
        return _NC_CACHE[key]

    nc = bacc.Bacc("TRN2", target_bir_lowering=False, debug=False,
                   num_devices=N_CORES)
    rows = n_blocks * P
    x = nc.dram_tensor("x", [rows, V], mybir.dt.float32,
                       kind="ExternalInput").ap()
    out = nc.dram_tensor("out", [n_blocks, P, n_chunks], mybir.dt.float32,
                         kind="ExternalOutput").ap()

    with tile.TileContext(nc) as tc:
        with (
            tc.tile_pool(name="data", bufs=bufs_in) as dpool,
            tc.tile_pool(name="expo", bufs=bufs_out) as epool,
            tc.tile_pool(name="acc", bufs=2) as apool,
        ):
            for b in range(n_blocks):
                acc = apool.tile([P, n_chunks], mybir.dt.float32)
                for c in range(n_chunks):
                    t = dpool.tile([P, chunk], mybir.dt.float32)
                    nc.sync.dma_start(
                        t[:], x[b * P:(b + 1) * P, c * chunk:(c + 1) * chunk])
                    e = epool.tile([P, chunk], mybir.dt.float32)
                    nc.scalar.activation(
                        e[:], t[:], mybir.ActivationFunctionType.Exp,
                        accum_out=acc[:, c:c + 1])
                nc.sync.dma_start(out[b], acc[:])

    nc.compile()
    _NC_CACHE[key] = nc
    return nc


def _run_device(shards: np.ndarray, trace: bool = False):
    """shards: [8, rows_per_core, V] f32.  Returns (rowsum [8*rows_per_core]
    float64, exec_time_ns or None)."""
    from concourse.bass_utils import run_bass_kernel_spmd

    n_blocks = shards.shape[1] // P
    nc = _build_nc(n_blocks)
    in_maps = [{"x": np.ascontiguousarray(shards[i])} for i in range(N_CORES)]
    res = run_bass_kernel_spmd(nc, in_maps, core_ids=list(range(N_CORES)),
                               trace=trace)
    outs = np.stack([res.results[i]["out"] for i in range(N_CORES)])
    # [8, n_blocks, P, n_chunks] -> per-row sums
    rowsum = outs.astype(np.float64).sum(axis=-1).reshape(-1)
    return rowsum, res.exec_time_ns


def kernel(output, trg, lengths):
    output = np.asarray(output, dtype=np.float32)
    trg = np.asarray(trg)
    lengths = np.asarray(lengths).astype(np.int64)

    tgt = trg[:, 1:]
    pos_valid = np.arange(S)[None, :] < lengths[:, None]
    valid = pos_valid & (tgt != 0)
    n_valid = int(valid.sum())
    if n_valid == 0:
        return np.array(0.0, dtype=np.float32)

    rb, rs = np.nonzero(valid)
    flat = output.reshape(B * SP1, V)           # contiguous view, no copy
    row_idx = rb * SP1 + (rs + 1)               # skip BOS position
    tgt_vals = tgt[rb, rs].astype(np.int64)
    x_t = flat[row_idx, tgt_vals].astype(np.float64)   # gathered target logits

    rows_per_core = max(1, math.ceil(n_valid / (N_CORES * P))) * P
    total = rows_per_core * N_CORES
    packed = np.zeros((total, V), dtype=np.float32)
    np.take(flat, row_idx, axis=0, out=packed[:n_valid])

    rowsum, _ = _run_device(packed.reshape(N_CORES, rows_per_core, V))
    log_z = np.log(rowsum[:n_valid])

    loss = (log_z.sum() - x_t.sum()) / n_valid
    return np.array(loss, dtype=np.float32)
